# revision 5
# baseline (speedup 1.0000x reference)
"""Trainium2 Bass kernel for KNN-upsample (MLP on down points + KNN mean + residual).

Contract: kernel(**inputs) takes FULL numpy inputs (as produced by
setup_inputs) and returns the FULL (LU, N, D_OUT) float32 output.

Sharding: batch axis N=8 -> one NeuronCore per batch column (data
parallel, no cross-core communication).

Per-core device program:
  Phase A (MLP, bf16): h2x[m, :] = relu(down[m, :] @ W1 + b1) @ (W2/3)
    computed per 512-token PSUM stripe, stored bf16 to a DRAM scratch
    (256 B rows).
  Phase B (gather + combine): per chunk of C points one SWDGE dma_gather
    pulls K*C rows (256 B each) from the DRAM scratch.  Gathers
    round-robin SWDGE queues 0..3 so descriptor generation runs on all
    four Q7 core pairs concurrently (single queue is 4x slower; the
    transposing SBUF-source gather corrupts data when queues overlap -
    shared XBAR - so the plain CME path is used).  Slots are ordered
    j = (k*upc + u)*128 + p so each k-plane lands contiguous per
    partition; DVE sums the three planes plus the (bias-folded) up
    features.

Host-side preprocessing (pure data-layout / index work):
  - down column transposed to (D_IN, LD) and cast bf16
  - W1, W2/3 cast bf16 (the /3 folds the k-mean); b2 folded into up
  - up cast bf16, packed chunk-major [c, p, u, f]
  - idx cast int16, slot-ordered, wrapped into the [16, ni/16] SWDGE
    layout and replicated x8 across partitions
  - out stored chunk-major [c, p, u, f]; host applies the inverse
    permutation
"""

import numpy as np
import ml_dtypes
from contextlib import ExitStack

import concourse.bacc as bacc
import concourse.tile as tile
import concourse.mybir as mybir
from concourse.bass_utils import run_bass_kernel_spmd

LD, LU, N, D_IN, D_OUT, K = 16384, 65536, 8, 256, 128, 3

CHUNK = 2048            # upsample points per gather chunk
MLP_BLOCK = 2048        # down points per MLP dma block
NCORES = 8
NQUEUES = 4             # SWDGE queues used round-robin by the gathers

F32 = mybir.dt.float32
BF16 = mybir.dt.bfloat16
I16 = mybir.dt.int16

_BUILD_CACHE = {}


def _build(ld=LD, lu=LU, d_in=D_IN, d_out=D_OUT, chunk=CHUNK, mlp_block=MLP_BLOCK):
    """Build + compile the per-core Bass program (identical on all cores)."""
    key = (ld, lu, d_in, d_out, chunk, mlp_block)
    if key in _BUILD_CACHE:
        return _BUILD_CACHE[key]

    nchunk = lu // chunk
    upc = chunk // 128                  # 128-point groups per chunk
    ni = K * chunk                      # gather indices per chunk
    nkb = d_in // 128                   # contraction tiles for matmul 1
    nblk = ld // mlp_block              # MLP dma blocks
    spb = mlp_block // 128              # 128-token sub-tiles per MLP block

    nc = bacc.Bacc("TRN2", target_bir_lowering=False, debug=False,
                   num_swdge_queues=NQUEUES)

    downt_d = nc.dram_tensor("downt", (d_in, ld), BF16, kind="ExternalInput")
    w1_d = nc.dram_tensor("w1", (d_in, d_out), BF16, kind="ExternalInput")
    b1_d = nc.dram_tensor("b1", (d_out, 1), F32, kind="ExternalInput")
    w2s_d = nc.dram_tensor("w2s", (d_out, d_out), BF16, kind="ExternalInput")
    upb_d = nc.dram_tensor("upb", (nchunk, 128, upc, d_out), BF16,
                           kind="ExternalInput")
    idx_d = nc.dram_tensor("idxp", (nchunk, 128, ni // 16), I16,
                           kind="ExternalInput")
    out_d = nc.dram_tensor("out", (nchunk, 128, upc, d_out), F32,
                           kind="ExternalOutput")
    h2x_d = nc.dram_tensor("h2x", (ld, d_out), F32, kind="Internal")

    relu = mybir.ActivationFunctionType.Relu

    # h2x viewed so a whole MLP block stores with one DMA:
    # row index m = b*mlp_block + s*128 + p  ->  [b, p, s, f]
    h2x_blk = h2x_d.ap().rearrange("(b s p) f -> b p s f", b=nblk, s=spb, p=128)

    with tile.TileContext(nc) as tc, ExitStack() as ctx:
        consts = ctx.enter_context(tc.tile_pool(name="consts", bufs=1))
        w1_t = consts.tile([128, nkb, d_out], BF16)
        w1_v = w1_d.ap().rearrange("(j p) e -> j p e", j=nkb, p=128)
        for j in range(nkb):
            nc.sync.dma_start(w1_t[:, j, :], w1_v[j])
        w2s_t = consts.tile([128, d_out], BF16)
        nc.sync.dma_start(w2s_t[:], w2s_d.ap())
        b1_t = consts.tile([128, 1], F32)
        nc.sync.dma_start(b1_t[:], b1_d.ap())

        # ---------------- Phase A: MLP ----------------
        stripe = 512                      # PSUM bank / max moving free dim
        with tc.tile_pool(name="dn", bufs=3) as dn_pool, \
             tc.tile_pool(name="ps1", bufs=2, space="PSUM") as ps1_pool, \
             tc.tile_pool(name="hT", bufs=2) as hT_pool, \
             tc.tile_pool(name="ps2", bufs=4, space="PSUM") as ps2_pool, \
             tc.tile_pool(name="h2b", bufs=2) as h2b_pool:
            for b in range(nblk):
                dn = dn_pool.tile([128, nkb, mlp_block], BF16)
                for j in range(nkb):
                    nc.sync.dma_start(
                        dn[:, j, :],
                        downt_d.ap()[j * 128:(j + 1) * 128,
                                     b * mlp_block:(b + 1) * mlp_block])
                hT = hT_pool.tile([128, mlp_block], BF16)
                for t0 in range(0, mlp_block, stripe):
                    ps1 = ps1_pool.tile([128, stripe], F32)
                    for j in range(nkb):
                        nc.tensor.matmul(ps1[:], w1_t[:, j, :],
                                         dn[:, j, t0:t0 + stripe],
                                         start=(j == 0), stop=(j == nkb - 1))
                    nc.scalar.activation(hT[:, t0:t0 + stripe], ps1[:], relu,
                                         bias=b1_t[:])
                h2b = h2b_pool.tile([128, spb, d_out], F32)
                for s in range(spb):
                    ps2 = ps2_pool.tile([128, d_out], F32)
                    nc.tensor.matmul(ps2[:], hT[:, s * 128:(s + 1) * 128],
                                     w2s_t[:], start=True, stop=True)
                    nc.vector.tensor_copy(h2b[:, s, :], ps2[:])
                nc.sync.dma_start(h2x_blk[b], h2b[:])

        # ---------------- Phase B: gather + combine ----------------
        with tc.tile_pool(name="gat", bufs=NQUEUES) as g_pool, \
             tc.tile_pool(name="ix", bufs=NQUEUES) as ix_pool, \
             tc.tile_pool(name="upt", bufs=3) as up_pool, \
             tc.tile_pool(name="tt", bufs=3) as t_pool, \
             tc.tile_pool(name="ot", bufs=3) as o_pool:
            for c in range(nchunk):
                ix = ix_pool.tile([128, ni // 16], I16)
                nc.sync.dma_start(ix[:], idx_d.ap()[c])
                g = g_pool.tile([128, K, upc, d_out], F32)
                nc.gpsimd.dma_gather(
                    g[:].rearrange("p k u f -> p (k u) f"),
                    h2x_d.ap(), ix[:],
                    num_idxs=ni, num_idxs_reg=ni, elem_size=d_out,
                    single_packet=False, queue_num=c % NQUEUES)
                upt = up_pool.tile([128, upc, d_out], BF16)
                nc.sync.dma_start(upt[:], upb_d.ap()[c])
                t = t_pool.tile([128, upc, d_out], F32)
                nc.vector.tensor_add(t[:], g[:, 0], g[:, 1])
                nc.vector.tensor_add(t[:], t[:], g[:, 2])
                o = o_pool.tile([128, upc, d_out], F32)
                nc.vector.tensor_add(o[:], t[:], upt[:])
                nc.sync.dma_start(out_d.ap()[c], o[:])

    nc.compile()
    _BUILD_CACHE[key] = nc
    return nc


def _prep_core_inputs(down_features, up_features, idx, W1, b1, W2, b2, n,
                      ld=LD, lu=LU, d_in=D_IN, d_out=D_OUT, chunk=CHUNK):
    """Host-side packing of the full inputs into core n's input map."""
    nchunk = lu // chunk
    upc = chunk // 128
    ni = K * chunk

    downt = np.ascontiguousarray(
        down_features[:, n, :].T).astype(ml_dtypes.bfloat16)
    upb = up_features[:, n, :].astype(np.float32) + b2[None, :].astype(np.float32)
    # (lu, d_out) -> [c, u, p, f] -> [c, p, u, f]
    upb = np.ascontiguousarray(
        upb.reshape(nchunk, upc, 128, d_out).transpose(0, 2, 1, 3)
    ).astype(ml_dtypes.bfloat16)

    idxn = idx[:, n, :].astype(np.int16)            # (lu, K)
    # point i = c*chunk + 128*u + p, neighbor k -> slot j = (k*upc + u)*128 + p
    perm = idxn.reshape(nchunk, upc, 128, K).transpose(0, 3, 1, 2)  # [c, k, u, p]
    flat = perm.reshape(nchunk, ni)                                  # slot-major
    wrapped = flat.reshape(nchunk, ni // 16, 16).transpose(0, 2, 1)  # [c, 16, ni/16]
    idxp = np.ascontiguousarray(np.tile(wrapped, (1, 8, 1)))         # [c, 128, ni/16]

    return {
        "downt": downt,
        "w1": np.ascontiguousarray(W1).astype(ml_dtypes.bfloat16),
        "b1": np.ascontiguousarray(b1.astype(np.float32).reshape(d_out, 1)),
        "w2s": np.ascontiguousarray(W2.astype(np.float32)
                                    / np.float32(K)).astype(ml_dtypes.bfloat16),
        "upb": upb,
        "idxp": idxp,
    }


def _unpack_out(out_np, lu=LU, d_out=D_OUT, chunk=CHUNK):
    nchunk = lu // chunk
    upc = chunk // 128
    return np.ascontiguousarray(
        out_np.reshape(nchunk, 128, upc, d_out).transpose(0, 2, 1, 3)
    ).reshape(lu, d_out)


def kernel(down_features, up_features, idx, W1, b1, W2, b2):
    down_features = np.asarray(down_features)
    up_features = np.asarray(up_features)
    idx = np.asarray(idx)
    W1, b1, W2, b2 = (np.asarray(a) for a in (W1, b1, W2, b2))

    nc = _build()
    in_maps = [
        _prep_core_inputs(down_features, up_features, idx, W1, b1, W2, b2, n)
        for n in range(NCORES)
    ]
    res = run_bass_kernel_spmd(nc, in_maps, core_ids=list(range(NCORES)))
    cols = [_unpack_out(res.results[n]["out"]) for n in range(NCORES)]
    return np.stack(cols, axis=1).astype(np.float32)


# revision 6
# speedup vs baseline: 1.2970x; 1.2970x over previous
"""Trainium2 Bass kernel for KNN-upsample (MLP on down points + KNN mean + residual).

Contract: kernel(**inputs) takes FULL numpy inputs (as produced by
setup_inputs) and returns the FULL (LU, N, D_OUT) float32 output.

Sharding: batch axis N=8 -> one NeuronCore per batch column (data
parallel, no cross-core communication).

Per-core device program:
  Phase A (MLP, bf16): h2x[m, :] = relu(down[m, :] @ W1 + b1) @ (W2/3)
    computed per 512-token PSUM stripe, stored bf16 to a DRAM scratch
    (256 B rows).
  Phase B (gather + combine): per chunk of C points one SWDGE dma_gather
    pulls K*C rows (256 B each) from the DRAM scratch.  Gathers
    round-robin SWDGE queues 0..3 so descriptor generation runs on all
    four Q7 core pairs concurrently (single queue is 4x slower; the
    transposing SBUF-source gather corrupts data when queues overlap -
    shared XBAR - so the plain CME path is used).  Slots are ordered
    j = (k*upc + u)*128 + p so each k-plane lands contiguous per
    partition; DVE sums the three planes plus the (bias-folded) up
    features.

Host-side preprocessing (pure data-layout / index work):
  - down column transposed to (D_IN, LD) and cast bf16
  - W1, W2/3 cast bf16 (the /3 folds the k-mean); b2 folded into up
  - up cast bf16, packed chunk-major [c, p, u, f]
  - idx cast int16, slot-ordered, wrapped into the [16, ni/16] SWDGE
    layout and replicated x8 across partitions
  - out stored chunk-major [c, p, u, f]; host applies the inverse
    permutation
"""

import numpy as np
import ml_dtypes
from contextlib import ExitStack

import concourse.bacc as bacc
import concourse.tile as tile
import concourse.mybir as mybir
from concourse.bass_utils import run_bass_kernel_spmd

LD, LU, N, D_IN, D_OUT, K = 16384, 65536, 8, 256, 128, 3

CHUNK = 2048            # upsample points per gather chunk
MLP_BLOCK = 2048        # down points per MLP dma block
NCORES = 8
NQUEUES = 4             # SWDGE queues used round-robin by the gathers

F32 = mybir.dt.float32
BF16 = mybir.dt.bfloat16
I16 = mybir.dt.int16

_BUILD_CACHE = {}


def _build(ld=LD, lu=LU, d_in=D_IN, d_out=D_OUT, chunk=CHUNK, mlp_block=MLP_BLOCK):
    """Build + compile the per-core Bass program (identical on all cores)."""
    key = (ld, lu, d_in, d_out, chunk, mlp_block)
    if key in _BUILD_CACHE:
        return _BUILD_CACHE[key]

    nchunk = lu // chunk
    upc = chunk // 128                  # 128-point groups per chunk
    ni = K * chunk                      # gather indices per chunk
    nkb = d_in // 128                   # contraction tiles for matmul 1
    nblk = ld // mlp_block              # MLP dma blocks
    spb = mlp_block // 128              # 128-token sub-tiles per MLP block

    nc = bacc.Bacc("TRN2", target_bir_lowering=False, debug=False,
                   num_swdge_queues=NQUEUES)

    downt_d = nc.dram_tensor("downt", (d_in, ld), BF16, kind="ExternalInput")
    w1_d = nc.dram_tensor("w1", (d_in, d_out), BF16, kind="ExternalInput")
    b1_d = nc.dram_tensor("b1", (d_out, 1), F32, kind="ExternalInput")
    w2s_d = nc.dram_tensor("w2s", (d_out, d_out), BF16, kind="ExternalInput")
    upb_d = nc.dram_tensor("upb", (nchunk, 128, upc, d_out), BF16,
                           kind="ExternalInput")
    idx_d = nc.dram_tensor("idxp", (nchunk, 128, ni // 16), I16,
                           kind="ExternalInput")
    out_d = nc.dram_tensor("out", (nchunk, 128, upc, d_out), F32,
                           kind="ExternalOutput")
    h2x_d = nc.dram_tensor("h2x", (ld, d_out), BF16, kind="Internal")

    relu = mybir.ActivationFunctionType.Relu

    # h2x viewed so a whole MLP block stores with one DMA:
    # row index m = b*mlp_block + s*128 + p  ->  [b, p, s, f]
    h2x_blk = h2x_d.ap().rearrange("(b s p) f -> b p s f", b=nblk, s=spb, p=128)

    with tile.TileContext(nc) as tc, ExitStack() as ctx:
        consts = ctx.enter_context(tc.tile_pool(name="consts", bufs=1))
        w1_t = consts.tile([128, nkb, d_out], BF16)
        w1_v = w1_d.ap().rearrange("(j p) e -> j p e", j=nkb, p=128)
        for j in range(nkb):
            nc.sync.dma_start(w1_t[:, j, :], w1_v[j])
        w2s_t = consts.tile([128, d_out], BF16)
        nc.sync.dma_start(w2s_t[:], w2s_d.ap())
        b1_t = consts.tile([128, 1], F32)
        nc.sync.dma_start(b1_t[:], b1_d.ap())

        # ---------------- Phase A: MLP ----------------
        stripe = 512                      # PSUM bank / max moving free dim
        with tc.tile_pool(name="dn", bufs=3) as dn_pool, \
             tc.tile_pool(name="ps1", bufs=2, space="PSUM") as ps1_pool, \
             tc.tile_pool(name="hT", bufs=2) as hT_pool, \
             tc.tile_pool(name="ps2", bufs=4, space="PSUM") as ps2_pool, \
             tc.tile_pool(name="h2b", bufs=2) as h2b_pool:
            for b in range(nblk):
                dn = dn_pool.tile([128, nkb, mlp_block], BF16)
                for j in range(nkb):
                    nc.sync.dma_start(
                        dn[:, j, :],
                        downt_d.ap()[j * 128:(j + 1) * 128,
                                     b * mlp_block:(b + 1) * mlp_block])
                hT = hT_pool.tile([128, mlp_block], BF16)
                for t0 in range(0, mlp_block, stripe):
                    ps1 = ps1_pool.tile([128, stripe], F32)
                    for j in range(nkb):
                        nc.tensor.matmul(ps1[:], w1_t[:, j, :],
                                         dn[:, j, t0:t0 + stripe],
                                         start=(j == 0), stop=(j == nkb - 1))
                    nc.scalar.activation(hT[:, t0:t0 + stripe], ps1[:], relu,
                                         bias=b1_t[:])
                h2b = h2b_pool.tile([128, spb, d_out], BF16)
                for s in range(spb):
                    ps2 = ps2_pool.tile([128, d_out], F32)
                    nc.tensor.matmul(ps2[:], hT[:, s * 128:(s + 1) * 128],
                                     w2s_t[:], start=True, stop=True)
                    nc.vector.tensor_copy(h2b[:, s, :], ps2[:])
                nc.sync.dma_start(h2x_blk[b], h2b[:])

        # ---------------- Phase B: gather + combine ----------------
        with tc.tile_pool(name="gat", bufs=2 * NQUEUES) as g_pool, \
             tc.tile_pool(name="ix", bufs=2 * NQUEUES) as ix_pool, \
             tc.tile_pool(name="upt", bufs=3) as up_pool, \
             tc.tile_pool(name="tt", bufs=3) as t_pool, \
             tc.tile_pool(name="ot", bufs=3) as o_pool:
            for c in range(nchunk):
                ix = ix_pool.tile([128, ni // 16], I16)
                nc.sync.dma_start(ix[:], idx_d.ap()[c])
                g = g_pool.tile([128, K, upc, d_out], BF16)
                nc.gpsimd.dma_gather(
                    g[:].rearrange("p k u f -> p (k u) f"),
                    h2x_d.ap(), ix[:],
                    num_idxs=ni, num_idxs_reg=ni, elem_size=d_out,
                    single_packet=False, queue_num=c % NQUEUES)
                upt = up_pool.tile([128, upc, d_out], BF16)
                nc.sync.dma_start(upt[:], upb_d.ap()[c])
                t = t_pool.tile([128, upc, d_out], F32)
                nc.vector.tensor_add(t[:], g[:, 0], g[:, 1])
                nc.vector.tensor_add(t[:], t[:], g[:, 2])
                o = o_pool.tile([128, upc, d_out], F32)
                nc.vector.tensor_add(o[:], t[:], upt[:])
                nc.sync.dma_start(out_d.ap()[c], o[:])

    nc.compile()
    _BUILD_CACHE[key] = nc
    return nc


def _prep_core_inputs(down_features, up_features, idx, W1, b1, W2, b2, n,
                      ld=LD, lu=LU, d_in=D_IN, d_out=D_OUT, chunk=CHUNK):
    """Host-side packing of the full inputs into core n's input map."""
    nchunk = lu // chunk
    upc = chunk // 128
    ni = K * chunk

    downt = np.ascontiguousarray(
        down_features[:, n, :].T).astype(ml_dtypes.bfloat16)
    upb = up_features[:, n, :].astype(np.float32) + b2[None, :].astype(np.float32)
    # (lu, d_out) -> [c, u, p, f] -> [c, p, u, f]
    upb = np.ascontiguousarray(
        upb.reshape(nchunk, upc, 128, d_out).transpose(0, 2, 1, 3)
    ).astype(ml_dtypes.bfloat16)

    idxn = idx[:, n, :].astype(np.int16)            # (lu, K)
    # point i = c*chunk + 128*u + p, neighbor k -> slot j = (k*upc + u)*128 + p
    perm = idxn.reshape(nchunk, upc, 128, K).transpose(0, 3, 1, 2)  # [c, k, u, p]
    flat = perm.reshape(nchunk, ni)                                  # slot-major
    wrapped = flat.reshape(nchunk, ni // 16, 16).transpose(0, 2, 1)  # [c, 16, ni/16]
    idxp = np.ascontiguousarray(np.tile(wrapped, (1, 8, 1)))         # [c, 128, ni/16]

    return {
        "downt": downt,
        "w1": np.ascontiguousarray(W1).astype(ml_dtypes.bfloat16),
        "b1": np.ascontiguousarray(b1.astype(np.float32).reshape(d_out, 1)),
        "w2s": np.ascontiguousarray(W2.astype(np.float32)
                                    / np.float32(K)).astype(ml_dtypes.bfloat16),
        "upb": upb,
        "idxp": idxp,
    }


def _unpack_out(out_np, lu=LU, d_out=D_OUT, chunk=CHUNK):
    nchunk = lu // chunk
    upc = chunk // 128
    return np.ascontiguousarray(
        out_np.reshape(nchunk, 128, upc, d_out).transpose(0, 2, 1, 3)
    ).reshape(lu, d_out)


def kernel(down_features, up_features, idx, W1, b1, W2, b2):
    down_features = np.asarray(down_features)
    up_features = np.asarray(up_features)
    idx = np.asarray(idx)
    W1, b1, W2, b2 = (np.asarray(a) for a in (W1, b1, W2, b2))

    nc = _build()
    in_maps = [
        _prep_core_inputs(down_features, up_features, idx, W1, b1, W2, b2, n)
        for n in range(NCORES)
    ]
    res = run_bass_kernel_spmd(nc, in_maps, core_ids=list(range(NCORES)))
    cols = [_unpack_out(res.results[n]["out"]) for n in range(NCORES)]
    return np.stack(cols, axis=1).astype(np.float32)
